# revision 10
# baseline (speedup 1.0000x reference)
"""GraphSAGE-style 2-layer GNN + per-graph readout on 8 Trainium2 NeuronCores.

Sharding: nodes split into 8 shards of 12500; each core owns the edges whose
dst lies in its shard (~75K/core). feat is replicated to every core, so layer-1
messages need no halo exchange; layer 2 is transform-first (z2 = h1 @ Wneigh2,
[N, 20]) with one 8MB AllGather, split in 4 so it overlaps layer-1 compute.

Per dst-tile (128 nodes) aggregation:
  - per-edge messages fetched by indirect DMA, 128 rows (one chunk) per call
  - one-hot selection matrices S[e, j] = inv_deg[dst_e] * [dst_e == j] built
    on DVE in a single tensor_scalar op (is_equal then mult) — mean-scaling is
    folded into S, so the PSUM result needs no post-scaling
  - layer 1 accumulates transposed (lhsT=msgs): PSUM gets aggT [dims, nodes],
    which feeds the transform matmuls directly as their moving operand
  - layer 2 accumulates natural (lhsT=S): PSUM gets agg2 [nodes, 20]; the
    self term s2 (+b2) is added into the same PSUM with an identity matmul,
    and ReLU reads the PSUM directly.

Nodes are assigned to tiles by a degree-balanced snake deal so every tile has
~765 edges -> chunk count is near the 586/core minimum (no padding blowup).
All FLOPs are f32 with f32 PSUM accumulation (matches reference to ~1e-6).
"""

import numpy as np
from contextlib import ExitStack

import concourse.bass as bass
import concourse.tile as tile
from concourse import bacc, mybir
from concourse.bass import IndirectOffsetOnAxis
from concourse.bass_utils import run_bass_kernel_spmd

N_NODES = 100000
N_EDGES = 600000
N_GRAPHS = 64
DIN = 128
D1 = 100
D2 = 20
N_CORES = 8
NLOC = N_NODES // N_CORES  # 12500
P = 128
NTILES = (NLOC + P - 1) // P  # 98
NPAD = NTILES * P  # 12544
QS = NPAD // 8  # 1568 rows per AllGather slice
RTOT = N_CORES * NPAD  # 100352 rows in the permuted/padded global tables

LAST_RESULTS = None  # set by kernel(); harnesses may read .exec_time_ns


def _host_prep(src, dst, graph_ids, feat):
    """Degree-balanced slot assignment + per-core edge/index arrays.

    Slot space: core c owns slots [0, NPAD); slot = tile*128 + lane. The
    feat gather table is uploaded in slot-global order (row c*NPAD + slot),
    the z2 table in quarter-major order (row q*8*QS + c*QS + (slot % QS)).
    """
    deg_global = np.bincount(dst, minlength=N_NODES).astype(np.int64)

    slot_of = np.empty(N_NODES, np.int64)
    node_at = np.full((N_CORES, NPAD), -1, np.int64)  # global node id or -1
    for c in range(N_CORES):
        loc_deg = deg_global[c * NLOC : (c + 1) * NLOC]
        order = np.argsort(-loc_deg, kind="stable")  # local ids, deg desc
        # tile 0 absorbs the 128 highest-degree nodes so the remaining tiles'
        # edge loads stay under 768 (= 6 chunks) even for edge-rich cores.
        snake = np.full((P, NTILES), -1, np.int64)
        snake[:, 0] = order[:P]
        arr = np.full((NTILES - 1) * P, -1, np.int64)
        arr[: NLOC - P] = order[P:]
        fold = arr.reshape(P, NTILES - 1)
        fold[1::2] = fold[1::2, ::-1]
        snake[:, 1:] = fold
        # snake[p, t] = local node at (tile t, lane p)
        valid = snake >= 0
        gids = np.where(valid, snake + c * NLOC, -1)
        node_at[c] = (gids.T).reshape(-1)  # slot = t*128 + p
        sl = np.arange(NPAD).reshape(NTILES, P).T  # sl[p, t] = t*128+p
        slot_of[gids[valid]] = sl[valid]

    # feat-table row for a global node (slot-global order)
    rowF = (np.arange(N_NODES) // NLOC) * NPAD + slot_of
    # z2-table row (quarter-major order)
    q = slot_of // QS
    rowZ = q * (N_CORES * QS) + (np.arange(N_NODES) // NLOC) * QS + (slot_of % QS)

    owner = dst // NLOC
    counts = np.zeros((N_CORES, NTILES), dtype=np.int64)
    per_core = []
    for c in range(N_CORES):
        m = owner == c
        s_c = src[m].astype(np.int64)
        dslot = slot_of[dst[m]]
        t_c = dslot // P
        np.add.at(counts[c], t_c, 1)
        per_core.append((s_c, dslot, t_c))

    K = np.maximum(1, (np.max(counts, axis=0) + P - 1) // P).astype(np.int64)
    offs = np.concatenate([[0], np.cumsum(K)])
    NCH = int(offs[-1])

    out = []
    for c in range(N_CORES):
        s_c, dslot, t_c = per_core[c]
        order = np.argsort(t_c, kind="stable")
        s_c, dslot, t_c = s_c[order], dslot[order], t_c[order]
        tile_starts = np.searchsorted(t_c, np.arange(NTILES))
        rank = np.arange(len(t_c)) - tile_starts[t_c]
        slot = offs[t_c] * P + rank

        gidxF = np.zeros((NCH * P,), dtype=np.int32)
        gidxZ = np.zeros((NCH * P,), dtype=np.int32)
        dstl = np.full((NCH * P,), -1.0, dtype=np.float32)
        degE = np.ones((NCH * P,), dtype=np.float32)
        gidxF[slot] = rowF[s_c]
        gidxZ[slot] = rowZ[s_c]
        dstl[slot] = (dslot % P).astype(np.float32)
        degE[slot] = deg_global[dst[owner == c][order]].astype(np.float32)

        def wrap(a):
            return np.ascontiguousarray(a.reshape(NCH, P).T)

        glT = np.where(node_at[c] >= 0,
                       graph_ids[np.maximum(node_at[c], 0)].astype(np.float32),
                       -1.0).astype(np.float32)
        flocT = np.zeros((DIN, NPAD), dtype=np.float32)
        valid = node_at[c] >= 0
        flocT[:, valid] = feat[node_at[c][valid]].T

        out.append({
            "gidxF": wrap(gidxF), "gidxZ": wrap(gidxZ), "dstl": wrap(dstl),
            "degE": wrap(degE),
            "glT": np.ascontiguousarray(glT.reshape(NTILES, P).T),
            "flocT": flocT,
        })

    featP = np.zeros((RTOT, DIN), dtype=np.float32)
    for c in range(N_CORES):
        valid = node_at[c] >= 0
        featP[c * NPAD + np.nonzero(valid)[0]] = feat[node_at[c][valid]]

    return K, offs, NCH, out, featP


def _build_program(K, offs, NCH):
    f32 = mybir.dt.float32
    AF = mybir.ActivationFunctionType
    nc = bacc.Bacc("TRN2", target_bir_lowering=False, debug=False,
                   num_devices=N_CORES)

    featP_in = nc.dram_tensor("featP", [RTOT, DIN], f32, kind="ExternalInput").ap()
    flocT_in = nc.dram_tensor("flocT", [DIN, NPAD], f32, kind="ExternalInput").ap()
    gidxF_in = nc.dram_tensor("gidxF", [P, NCH], mybir.dt.int32, kind="ExternalInput").ap()
    gidxZ_in = nc.dram_tensor("gidxZ", [P, NCH], mybir.dt.int32, kind="ExternalInput").ap()
    dstl_in = nc.dram_tensor("dstl", [P, NCH], f32, kind="ExternalInput").ap()
    degE_in = nc.dram_tensor("degE", [P, NCH], f32, kind="ExternalInput").ap()
    glT_in = nc.dram_tensor("glT", [P, NTILES], f32, kind="ExternalInput").ap()
    iota128_in = nc.dram_tensor("iota128", [P, P], mybir.dt.bfloat16, kind="ExternalInput").ap()
    iota64_in = nc.dram_tensor("iota64", [P, N_GRAPHS], f32, kind="ExternalInput").ap()
    ident_in = nc.dram_tensor("ident", [P, P], f32, kind="ExternalInput").ap()
    onesrow_in = nc.dram_tensor("onesrow", [1, P], f32, kind="ExternalInput").ap()
    ws1_in = nc.dram_tensor("ws1", [DIN, D1], f32, kind="ExternalInput").ap()
    wn1_in = nc.dram_tensor("wn1", [DIN, D1], f32, kind="ExternalInput").ap()
    b1c_in = nc.dram_tensor("b1c", [P, 1], f32, kind="ExternalInput").ap()
    ws2_in = nc.dram_tensor("ws2", [D1, D2], f32, kind="ExternalInput").ap()
    wn2_in = nc.dram_tensor("wn2", [D1, D2], f32, kind="ExternalInput").ap()
    b2_in = nc.dram_tensor("b2r", [1, D2], f32, kind="ExternalInput").ap()
    fc1w_in = nc.dram_tensor("fc1w", [D2, 10], f32, kind="ExternalInput").ap()
    fc1b_in = nc.dram_tensor("fc1b", [1, 10], f32, kind="ExternalInput").ap()
    fc2w_in = nc.dram_tensor("fc2w", [10, 1], f32, kind="ExternalInput").ap()
    fc2b_in = nc.dram_tensor("fc2b", [1, 1], f32, kind="ExternalInput").ap()
    out_dram = nc.dram_tensor("out", [N_GRAPHS, 1], f32, kind="ExternalOutput").ap()

    with tile.TileContext(nc) as tc, ExitStack() as ctx:
        consts = ctx.enter_context(tc.tile_pool(name="consts", bufs=1))
        msgs = ctx.enter_context(tc.tile_pool(name="msgs", bufs=16))
        sbuf = ctx.enter_context(tc.tile_pool(name="sbuf", bufs=3))
        keep = ctx.enter_context(tc.tile_pool(name="keep", bufs=1))
        dram = ctx.enter_context(tc.tile_pool(name="dram", bufs=1, space="DRAM"))
        ps_agg = ctx.enter_context(tc.tile_pool(name="ps_agg", bufs=3, space="PSUM"))
        ps_h = ctx.enter_context(tc.tile_pool(name="ps_h", bufs=2, space="PSUM"))
        ps_z = ctx.enter_context(tc.tile_pool(name="ps_z", bufs=1, space="PSUM"))
        ps_ro = ctx.enter_context(tc.tile_pool(name="ps_ro", bufs=1, space="PSUM"))

        def cload(ap_in, shape, name, dt=f32):
            t = consts.tile(shape, dt, tag=name)
            nc.sync.dma_start(t[:], ap_in)
            return t

        gidxF = cload(gidxF_in[:], [P, NCH], "gidxF", mybir.dt.int32)
        gidxZ = cload(gidxZ_in[:], [P, NCH], "gidxZ", mybir.dt.int32)
        dstl = cload(dstl_in[:], [P, NCH], "dstl")
        degE = cload(degE_in[:], [P, NCH], "degE")
        glT = cload(glT_in[:], [P, NTILES], "glT")
        iota128 = cload(iota128_in[:], [P, P], "iota128", mybir.dt.bfloat16)
        iota64 = cload(iota64_in[:], [P, N_GRAPHS], "iota64")
        ident = cload(ident_in[:], [P, P], "ident")
        onesrow = cload(onesrow_in[:], [1, P], "onesrow")
        ws1 = cload(ws1_in[:], [DIN, D1], "ws1")
        wn1 = cload(wn1_in[:], [DIN, D1], "wn1")
        b1c = cload(b1c_in[:], [P, 1], "b1c")
        ws2 = cload(ws2_in[:], [D1, D2], "ws2")
        wn2 = cload(wn2_in[:], [D1, D2], "wn2")
        b2r = cload(b2_in[:], [1, D2], "b2r")
        fc1w = cload(fc1w_in[:], [D2, 10], "fc1w")
        fc1b = cload(fc1b_in[:], [1, 10], "fc1b")
        fc2w = cload(fc2w_in[:], [10, 1], "fc2w")
        fc2b = cload(fc2b_in[:], [1, 1], "fc2b")

        # invdegE = 1 / max(degE, 1): per-edge-slot scale folded into S
        invdegE = keep.tile([P, NCH], f32, tag="invdegE")
        nc.vector.tensor_scalar_max(invdegE[:], degE[:], 1.0)
        nc.vector.reciprocal(invdegE[:], invdegE[:])

        s2_all = keep.tile([P, NTILES * D2], f32, tag="s2_all")
        z2_shard = dram.tile([NPAD, D2], f32, tag="z2_shard")
        z2_full = dram.tile([RTOT, D2], f32, tag="z2_full")
        ro_part = dram.tile([N_GRAPHS, D2 + 1], f32, tag="ro_part")
        ro_red = dram.tile([N_GRAPHS, D2 + 1], f32, tag="ro_red")

        def build_S(ch):
            s = msgs.tile([P, P], mybir.dt.bfloat16, tag="S")
            nc.vector.tensor_scalar(
                out=s[:], in0=iota128[:],
                scalar1=dstl[:, ch : ch + 1], scalar2=invdegE[:, ch : ch + 1],
                op0=mybir.AluOpType.is_equal, op1=mybir.AluOpType.mult,
            )
            return s

        # ---------- Layer 1 (aggT accumulation) + layer-2 transform ----------
        for t in range(NTILES):
            nch_t = int(K[t])
            paggT = ps_agg.tile([P, P], f32, tag="agg")
            for j in range(nch_t):
                ch = int(offs[t]) + j
                m = msgs.tile([P, DIN], mybir.dt.bfloat16, tag="msg1")
                nc.gpsimd.indirect_dma_start(
                    out=m[:], out_offset=None, in_=featP_in[:, :],
                    in_offset=IndirectOffsetOnAxis(ap=gidxF[:, ch : ch + 1], axis=0),
                )
                s = build_S(ch)
                nc.tensor.matmul(paggT[:], lhsT=m[:], rhs=s[:],
                                 start=(j == 0), stop=(j == nch_t - 1))
            aggT = sbuf.tile([P, P], f32, tag="aggT")
            nc.vector.tensor_copy(aggT[:], paggT[:])

            featT = sbuf.tile([P, P], f32, tag="featT")
            nc.sync.dma_start(featT[:], flocT_in[:, t * P : (t + 1) * P])

            ph1 = ps_h.tile([D1, P], f32, tag="h1T")
            nc.tensor.matmul(ph1[:], lhsT=ws1[:], rhs=featT[:], start=True, stop=False)
            nc.tensor.matmul(ph1[:], lhsT=wn1[:], rhs=aggT[:], start=False, stop=True)
            h1T = sbuf.tile([D1, P], f32, tag="h1T_sb")
            nc.scalar.activation(h1T[:], ph1[:], AF.Relu, bias=b1c[:D1, :])

            pz2 = ps_z.tile([P, D2], f32, tag="z2")
            nc.tensor.matmul(pz2[:], lhsT=h1T[:], rhs=wn2[:], start=True, stop=True)
            z2sb = sbuf.tile([P, D2], f32, tag="z2sb")
            nc.vector.tensor_copy(z2sb[:], pz2[:])
            nc.sync.dma_start(z2_shard[t * P : (t + 1) * P, :], z2sb[:])

            ps2 = ps_z.tile([P, D2], f32, tag="s2")
            nc.tensor.matmul(ps2[:], lhsT=h1T[:], rhs=ws2[:], start=True, stop=False)
            nc.tensor.matmul(ps2[:], lhsT=onesrow[:], rhs=b2r[:], start=False, stop=True)
            nc.scalar.copy(s2_all[:, t * D2 : (t + 1) * D2], ps2[:])

            # split AllGather: fire quarter q as soon as its rows are written
            for qq, tq in enumerate((12, 24, 36, 48, 61, 73, 85, NTILES - 1)):
                if t == tq:
                    nc.gpsimd.collective_compute(
                        "AllGather",
                        mybir.AluOpType.bypass,
                        ins=[z2_shard[qq * QS : (qq + 1) * QS, :]],
                        outs=[z2_full[qq * N_CORES * QS : (qq + 1) * N_CORES * QS, :]],
                        replica_groups=[list(range(N_CORES))],
                    )

        # ---------- Layer 2 + readout accumulation ----------
        pro = ps_ro.tile([N_GRAPHS, D2 + 1], f32, tag="ro")
        for t in range(NTILES):
            nch_t = int(K[t])
            pagg = ps_agg.tile([P, D2], f32, tag="agg")
            for j in range(nch_t):
                ch = int(offs[t]) + j
                m = msgs.tile([P, D2], mybir.dt.bfloat16, tag="msg2")
                nc.gpsimd.indirect_dma_start(
                    out=m[:], out_offset=None, in_=z2_full[:, :],
                    in_offset=IndirectOffsetOnAxis(ap=gidxZ[:, ch : ch + 1], axis=0),
                )
                s = build_S(ch)
                nc.tensor.matmul(pagg[:], lhsT=s[:], rhs=m[:],
                                 start=(j == 0), stop=False)
            # add self term s2 (+b2) into the same PSUM, then ReLU from PSUM
            nc.tensor.matmul(pagg[:], lhsT=ident[:],
                             rhs=s2_all[:, t * D2 : (t + 1) * D2],
                             start=False, stop=True)
            h2e = sbuf.tile([P, D2 + 1], f32, tag="h2e")
            nc.scalar.activation(h2e[:, :D2], pagg[:], AF.Relu)
            nc.vector.memset(h2e[:, D2 : D2 + 1], 1.0)

            sg = msgs.tile([P, N_GRAPHS], f32, tag="Sg")
            nc.vector.tensor_tensor(
                out=sg[:], in0=iota64[:],
                in1=glT[:, t : t + 1].to_broadcast([P, N_GRAPHS]),
                op=mybir.AluOpType.is_equal,
            )
            nc.tensor.matmul(pro[:], lhsT=sg[:], rhs=h2e[:],
                             start=(t == 0), stop=(t == NTILES - 1))

        ro_sb = sbuf.tile([N_GRAPHS, D2 + 1], f32, tag="ro_sb")
        nc.vector.tensor_copy(ro_sb[:], pro[:])
        nc.sync.dma_start(ro_part[:], ro_sb[:])
        nc.gpsimd.collective_compute(
            "AllReduce",
            mybir.AluOpType.add,
            ins=[ro_part.opt()],
            outs=[ro_red.opt()],
            replica_groups=[list(range(N_CORES))],
        )

        # ---------- Per-graph mean + MLP (redundant on every core) ----------
        ro2 = sbuf.tile([N_GRAPHS, D2 + 1], f32, tag="ro2")
        nc.sync.dma_start(ro2[:], ro_red[:])
        cinv = sbuf.tile([N_GRAPHS, 1], f32, tag="cinv")
        nc.vector.tensor_scalar_max(cinv[:], ro2[:, D2 : D2 + 1], 1.0)
        nc.vector.reciprocal(cinv[:], cinv[:])
        hg = sbuf.tile([N_GRAPHS, D2], f32, tag="hg")
        nc.vector.tensor_scalar(
            out=hg[:], in0=ro2[:, :D2], scalar1=cinv[:, 0:1], scalar2=None,
            op0=mybir.AluOpType.mult,
        )
        ptr4 = ps_agg.tile([P, P], f32, tag="agg")
        nc.tensor.transpose(ptr4[:D2, :N_GRAPHS], hg[:], ident[:N_GRAPHS, :N_GRAPHS])
        hgT = sbuf.tile([D2, N_GRAPHS], f32, tag="hgT")
        nc.vector.tensor_copy(hgT[:], ptr4[:D2, :N_GRAPHS])

        pfc1 = ps_h.tile([N_GRAPHS, 10], f32, tag="h1T")
        nc.tensor.matmul(pfc1[:], lhsT=hgT[:], rhs=fc1w[:], start=True, stop=False)
        nc.tensor.matmul(pfc1[:], lhsT=onesrow[:, :N_GRAPHS], rhs=fc1b[:], start=False, stop=True)
        a1 = sbuf.tile([N_GRAPHS, 10], f32, tag="a1")
        nc.scalar.activation(a1[:], pfc1[:], AF.Relu)

        ptr5 = ps_agg.tile([P, P], f32, tag="agg")
        nc.tensor.transpose(ptr5[:10, :N_GRAPHS], a1[:], ident[:N_GRAPHS, :N_GRAPHS])
        a1T = sbuf.tile([10, N_GRAPHS], f32, tag="a1T")
        nc.vector.tensor_copy(a1T[:], ptr5[:10, :N_GRAPHS])

        pout = ps_z.tile([N_GRAPHS, 1], f32, tag="z2")
        nc.tensor.matmul(pout[:], lhsT=a1T[:], rhs=fc2w[:], start=True, stop=False)
        nc.tensor.matmul(pout[:], lhsT=onesrow[:, :N_GRAPHS], rhs=fc2b[:], start=False, stop=True)
        osb = sbuf.tile([N_GRAPHS, 1], f32, tag="osb")
        nc.vector.tensor_copy(osb[:], pout[:])
        nc.sync.dma_start(out_dram[:], osb[:])

    nc.compile()
    return nc


def kernel(feat, Wself1, Wneigh1, b1, Wself2, Wneigh2, b2,
           fc1_w, fc1_b, fc2_w, fc2_b, src, dst, graph_ids):
    global LAST_RESULTS
    feat = np.asarray(feat, dtype=np.float32)
    src = np.asarray(src, dtype=np.int64)
    dst = np.asarray(dst, dtype=np.int64)
    graph_ids = np.asarray(graph_ids, dtype=np.int64)

    K, offs, NCH, percore, featP = _host_prep(src, dst, graph_ids, feat)
    nc = _build_program(K, offs, NCH)

    import ml_dtypes
    iota128 = np.tile(np.arange(P, dtype=np.float32), (P, 1)).astype(ml_dtypes.bfloat16)
    iota64 = np.tile(np.arange(N_GRAPHS, dtype=np.float32), (P, 1))
    ident = np.eye(P, dtype=np.float32)
    onesrow = np.ones((1, P), dtype=np.float32)
    b1c = np.zeros((P, 1), dtype=np.float32)
    b1c[:D1, 0] = np.asarray(b1, np.float32)

    in_maps = []
    for c in range(N_CORES):
        pc = percore[c]
        in_maps.append({
            "featP": featP,
            "flocT": pc["flocT"],
            "gidxF": pc["gidxF"],
            "gidxZ": pc["gidxZ"],
            "dstl": pc["dstl"],
            "degE": pc["degE"],
            "glT": pc["glT"],
            "iota128": iota128,
            "iota64": iota64,
            "ident": ident,
            "onesrow": onesrow,
            "ws1": np.asarray(Wself1, np.float32),
            "wn1": np.asarray(Wneigh1, np.float32),
            "b1c": b1c,
            "ws2": np.asarray(Wself2, np.float32),
            "wn2": np.asarray(Wneigh2, np.float32),
            "b2r": np.asarray(b2, np.float32).reshape(1, D2),
            "fc1w": np.asarray(fc1_w, np.float32),
            "fc1b": np.asarray(fc1_b, np.float32).reshape(1, 10),
            "fc2w": np.asarray(fc2_w, np.float32),
            "fc2b": np.asarray(fc2_b, np.float32).reshape(1, 1),
        })

    res = run_bass_kernel_spmd(nc, in_maps, list(range(N_CORES)))
    LAST_RESULTS = res
    return np.asarray(res.results[0]["out"], dtype=np.float32)


# revision 11
# speedup vs baseline: 1.0089x; 1.0089x over previous
"""GraphSAGE-style 2-layer GNN + per-graph readout on 8 Trainium2 NeuronCores.

Sharding: nodes split into 8 shards of 12500; each core owns the edges whose
dst lies in its shard (~75K/core). feat is replicated to every core, so layer-1
messages need no halo exchange; layer 2 is transform-first (z2 = h1 @ Wneigh2,
[N, 20]) with one 8MB AllGather, split in 4 so it overlaps layer-1 compute.

Per dst-tile (128 nodes) aggregation:
  - per-edge messages fetched by indirect DMA, 128 rows (one chunk) per call
  - one-hot selection matrices S[e, j] = inv_deg[dst_e] * [dst_e == j] built
    on DVE in a single tensor_scalar op (is_equal then mult) — mean-scaling is
    folded into S, so the PSUM result needs no post-scaling
  - layer 1 accumulates transposed (lhsT=msgs): PSUM gets aggT [dims, nodes],
    which feeds the transform matmuls directly as their moving operand
  - layer 2 accumulates natural (lhsT=S): PSUM gets agg2 [nodes, 20]; the
    self term s2 (+b2) is added into the same PSUM with an identity matmul,
    and ReLU reads the PSUM directly.

Nodes are assigned to tiles by a degree-balanced snake deal so every tile has
~765 edges -> chunk count is near the 586/core minimum (no padding blowup).
All FLOPs are f32 with f32 PSUM accumulation (matches reference to ~1e-6).
"""

import numpy as np
from contextlib import ExitStack

import concourse.bass as bass
import concourse.tile as tile
from concourse import bacc, mybir
from concourse.bass import IndirectOffsetOnAxis
from concourse.bass_utils import run_bass_kernel_spmd

N_NODES = 100000
N_EDGES = 600000
N_GRAPHS = 64
DIN = 128
D1 = 100
D2 = 20
N_CORES = 8
NLOC = N_NODES // N_CORES  # 12500
P = 128
NTILES = (NLOC + P - 1) // P  # 98
NPAD = NTILES * P  # 12544
QS = NPAD // 8  # 1568 rows per AllGather slice
RTOT = N_CORES * NPAD  # 100352 rows in the permuted/padded global tables

LAST_RESULTS = None  # set by kernel(); harnesses may read .exec_time_ns


def _host_prep(src, dst, graph_ids, feat):
    """Degree-balanced slot assignment + per-core edge/index arrays.

    Slot space: core c owns slots [0, NPAD); slot = tile*128 + lane. The
    feat gather table is uploaded in slot-global order (row c*NPAD + slot),
    the z2 table in quarter-major order (row q*8*QS + c*QS + (slot % QS)).
    """
    deg_global = np.bincount(dst, minlength=N_NODES).astype(np.int64)

    slot_of = np.empty(N_NODES, np.int64)
    node_at = np.full((N_CORES, NPAD), -1, np.int64)  # global node id or -1
    for c in range(N_CORES):
        loc_deg = deg_global[c * NLOC : (c + 1) * NLOC]
        order = np.argsort(-loc_deg, kind="stable")  # local ids, deg desc
        # tile 0 absorbs the 128 highest-degree nodes so the remaining tiles'
        # edge loads stay under 768 (= 6 chunks) even for edge-rich cores.
        snake = np.full((P, NTILES), -1, np.int64)
        snake[:, 0] = order[:P]
        arr = np.full((NTILES - 1) * P, -1, np.int64)
        arr[: NLOC - P] = order[P:]
        fold = arr.reshape(P, NTILES - 1)
        fold[1::2] = fold[1::2, ::-1]
        snake[:, 1:] = fold
        # snake[p, t] = local node at (tile t, lane p)
        valid = snake >= 0
        gids = np.where(valid, snake + c * NLOC, -1)
        node_at[c] = (gids.T).reshape(-1)  # slot = t*128 + p
        sl = np.arange(NPAD).reshape(NTILES, P).T  # sl[p, t] = t*128+p
        slot_of[gids[valid]] = sl[valid]

    # feat-table row for a global node (slot-global order)
    rowF = (np.arange(N_NODES) // NLOC) * NPAD + slot_of
    # z2-table row (quarter-major order)
    q = slot_of // QS
    rowZ = q * (N_CORES * QS) + (np.arange(N_NODES) // NLOC) * QS + (slot_of % QS)

    owner = dst // NLOC
    counts = np.zeros((N_CORES, NTILES), dtype=np.int64)
    per_core = []
    for c in range(N_CORES):
        m = owner == c
        s_c = src[m].astype(np.int64)
        dslot = slot_of[dst[m]]
        t_c = dslot // P
        np.add.at(counts[c], t_c, 1)
        per_core.append((s_c, dslot, t_c))

    K = np.maximum(1, (np.max(counts, axis=0) + P - 1) // P).astype(np.int64)
    offs = np.concatenate([[0], np.cumsum(K)])
    NCH = int(offs[-1])

    out = []
    for c in range(N_CORES):
        s_c, dslot, t_c = per_core[c]
        order = np.argsort(t_c, kind="stable")
        s_c, dslot, t_c = s_c[order], dslot[order], t_c[order]
        tile_starts = np.searchsorted(t_c, np.arange(NTILES))
        rank = np.arange(len(t_c)) - tile_starts[t_c]
        slot = offs[t_c] * P + rank

        gidxF = np.zeros((NCH * P,), dtype=np.int32)
        gidxZ = np.zeros((NCH * P,), dtype=np.int32)
        dstl = np.full((NCH * P,), -1.0, dtype=np.float32)
        degE = np.ones((NCH * P,), dtype=np.float32)
        gidxF[slot] = rowF[s_c]
        gidxZ[slot] = rowZ[s_c]
        dstl[slot] = (dslot % P).astype(np.float32)
        degE[slot] = deg_global[dst[owner == c][order]].astype(np.float32)

        def wrap(a):
            return np.ascontiguousarray(a.reshape(NCH, P).T)

        glT = np.where(node_at[c] >= 0,
                       graph_ids[np.maximum(node_at[c], 0)].astype(np.float32),
                       -1.0).astype(np.float32)
        flocT = np.zeros((DIN, NPAD), dtype=np.float32)
        valid = node_at[c] >= 0
        flocT[:, valid] = feat[node_at[c][valid]].T

        out.append({
            "gidxF": wrap(gidxF), "gidxZ": wrap(gidxZ), "dstl": wrap(dstl),
            "degE": wrap(degE),
            "glT": np.ascontiguousarray(glT.reshape(NTILES, P).T),
            "flocT": flocT,
        })

    featP = np.zeros((RTOT, DIN), dtype=np.float32)
    for c in range(N_CORES):
        valid = node_at[c] >= 0
        featP[c * NPAD + np.nonzero(valid)[0]] = feat[node_at[c][valid]]

    return K, offs, NCH, out, featP


def _build_program(K, offs, NCH):
    f32 = mybir.dt.float32
    AF = mybir.ActivationFunctionType
    nc = bacc.Bacc("TRN2", target_bir_lowering=False, debug=False,
                   num_devices=N_CORES)

    featP_in = nc.dram_tensor("featP", [RTOT, DIN], f32, kind="ExternalInput").ap()
    flocT_in = nc.dram_tensor("flocT", [DIN, NPAD], f32, kind="ExternalInput").ap()
    gidxF_in = nc.dram_tensor("gidxF", [P, NCH], mybir.dt.int32, kind="ExternalInput").ap()
    gidxZ_in = nc.dram_tensor("gidxZ", [P, NCH], mybir.dt.int32, kind="ExternalInput").ap()
    dstl_in = nc.dram_tensor("dstl", [P, NCH], f32, kind="ExternalInput").ap()
    degE_in = nc.dram_tensor("degE", [P, NCH], f32, kind="ExternalInput").ap()
    glT_in = nc.dram_tensor("glT", [P, NTILES], f32, kind="ExternalInput").ap()
    iota128_in = nc.dram_tensor("iota128", [P, P], f32, kind="ExternalInput").ap()
    iota64_in = nc.dram_tensor("iota64", [P, N_GRAPHS], f32, kind="ExternalInput").ap()
    ident_in = nc.dram_tensor("ident", [P, P], f32, kind="ExternalInput").ap()
    onesrow_in = nc.dram_tensor("onesrow", [1, P], f32, kind="ExternalInput").ap()
    ws1_in = nc.dram_tensor("ws1", [DIN, D1], f32, kind="ExternalInput").ap()
    wn1_in = nc.dram_tensor("wn1", [DIN, D1], f32, kind="ExternalInput").ap()
    b1c_in = nc.dram_tensor("b1c", [P, 1], f32, kind="ExternalInput").ap()
    ws2_in = nc.dram_tensor("ws2", [D1, D2], f32, kind="ExternalInput").ap()
    wn2_in = nc.dram_tensor("wn2", [D1, D2], f32, kind="ExternalInput").ap()
    b2_in = nc.dram_tensor("b2r", [1, D2], f32, kind="ExternalInput").ap()
    fc1w_in = nc.dram_tensor("fc1w", [D2, 10], f32, kind="ExternalInput").ap()
    fc1b_in = nc.dram_tensor("fc1b", [1, 10], f32, kind="ExternalInput").ap()
    fc2w_in = nc.dram_tensor("fc2w", [10, 1], f32, kind="ExternalInput").ap()
    fc2b_in = nc.dram_tensor("fc2b", [1, 1], f32, kind="ExternalInput").ap()
    out_dram = nc.dram_tensor("out", [N_GRAPHS, 1], f32, kind="ExternalOutput").ap()

    with tile.TileContext(nc) as tc, ExitStack() as ctx:
        consts = ctx.enter_context(tc.tile_pool(name="consts", bufs=1))
        msgs = ctx.enter_context(tc.tile_pool(name="msgs", bufs=16))
        sbuf = ctx.enter_context(tc.tile_pool(name="sbuf", bufs=3))
        keep = ctx.enter_context(tc.tile_pool(name="keep", bufs=1))
        dram = ctx.enter_context(tc.tile_pool(name="dram", bufs=1, space="DRAM"))
        ps_agg = ctx.enter_context(tc.tile_pool(name="ps_agg", bufs=3, space="PSUM"))
        ps_h = ctx.enter_context(tc.tile_pool(name="ps_h", bufs=2, space="PSUM"))
        ps_z = ctx.enter_context(tc.tile_pool(name="ps_z", bufs=1, space="PSUM"))
        ps_ro = ctx.enter_context(tc.tile_pool(name="ps_ro", bufs=1, space="PSUM"))

        def cload(ap_in, shape, name, dt=f32):
            t = consts.tile(shape, dt, tag=name)
            nc.sync.dma_start(t[:], ap_in)
            return t

        gidxF = cload(gidxF_in[:], [P, NCH], "gidxF", mybir.dt.int32)
        gidxZ = cload(gidxZ_in[:], [P, NCH], "gidxZ", mybir.dt.int32)
        dstl = cload(dstl_in[:], [P, NCH], "dstl")
        degE = cload(degE_in[:], [P, NCH], "degE")
        glT = cload(glT_in[:], [P, NTILES], "glT")
        iota128 = cload(iota128_in[:], [P, P], "iota128")
        iota64 = cload(iota64_in[:], [P, N_GRAPHS], "iota64")
        ident = cload(ident_in[:], [P, P], "ident")
        onesrow = cload(onesrow_in[:], [1, P], "onesrow")
        ws1 = cload(ws1_in[:], [DIN, D1], "ws1")
        wn1 = cload(wn1_in[:], [DIN, D1], "wn1")
        b1c = cload(b1c_in[:], [P, 1], "b1c")
        ws2 = cload(ws2_in[:], [D1, D2], "ws2")
        wn2 = cload(wn2_in[:], [D1, D2], "wn2")
        b2r = cload(b2_in[:], [1, D2], "b2r")
        fc1w = cload(fc1w_in[:], [D2, 10], "fc1w")
        fc1b = cload(fc1b_in[:], [1, 10], "fc1b")
        fc2w = cload(fc2w_in[:], [10, 1], "fc2w")
        fc2b = cload(fc2b_in[:], [1, 1], "fc2b")

        # invdegE = 1 / max(degE, 1): per-edge-slot scale folded into S
        invdegE = keep.tile([P, NCH], f32, tag="invdegE")
        nc.vector.tensor_scalar_max(invdegE[:], degE[:], 1.0)
        nc.vector.reciprocal(invdegE[:], invdegE[:])

        s2_all = keep.tile([P, NTILES * D2], f32, tag="s2_all")
        z2_shard = dram.tile([NPAD, D2], f32, tag="z2_shard")
        z2_full = dram.tile([RTOT, D2], f32, tag="z2_full")
        ro_part = dram.tile([N_GRAPHS, D2 + 1], f32, tag="ro_part")
        ro_red = dram.tile([N_GRAPHS, D2 + 1], f32, tag="ro_red")

        def build_S(ch):
            s = msgs.tile([P, P], f32, tag="S")
            nc.vector.tensor_scalar(
                out=s[:], in0=iota128[:],
                scalar1=dstl[:, ch : ch + 1], scalar2=invdegE[:, ch : ch + 1],
                op0=mybir.AluOpType.is_equal, op1=mybir.AluOpType.mult,
            )
            return s

        # ---------- Layer 1 (aggT accumulation) + layer-2 transform ----------
        for t in range(NTILES):
            nch_t = int(K[t])
            paggT = ps_agg.tile([P, P], f32, tag="agg")
            for j in range(nch_t):
                ch = int(offs[t]) + j
                m = msgs.tile([P, DIN], f32, tag="msg1")
                nc.gpsimd.indirect_dma_start(
                    out=m[:], out_offset=None, in_=featP_in[:, :],
                    in_offset=IndirectOffsetOnAxis(ap=gidxF[:, ch : ch + 1], axis=0),
                )
                s = build_S(ch)
                nc.tensor.matmul(paggT[:], lhsT=m[:], rhs=s[:],
                                 start=(j == 0), stop=(j == nch_t - 1))
            aggT = sbuf.tile([P, P], f32, tag="aggT")
            nc.vector.tensor_copy(aggT[:], paggT[:])

            featT = sbuf.tile([P, P], f32, tag="featT")
            nc.sync.dma_start(featT[:], flocT_in[:, t * P : (t + 1) * P])

            ph1 = ps_h.tile([D1, P], f32, tag="h1T")
            nc.tensor.matmul(ph1[:], lhsT=ws1[:], rhs=featT[:], start=True, stop=False)
            nc.tensor.matmul(ph1[:], lhsT=wn1[:], rhs=aggT[:], start=False, stop=True)
            h1T = sbuf.tile([D1, P], f32, tag="h1T_sb")
            nc.scalar.activation(h1T[:], ph1[:], AF.Relu, bias=b1c[:D1, :])

            pz2 = ps_z.tile([P, D2], f32, tag="z2")
            nc.tensor.matmul(pz2[:], lhsT=h1T[:], rhs=wn2[:], start=True, stop=True)
            z2sb = sbuf.tile([P, D2], f32, tag="z2sb")
            nc.vector.tensor_copy(z2sb[:], pz2[:])
            nc.sync.dma_start(z2_shard[t * P : (t + 1) * P, :], z2sb[:])

            ps2 = ps_z.tile([P, D2], f32, tag="s2")
            nc.tensor.matmul(ps2[:], lhsT=h1T[:], rhs=ws2[:], start=True, stop=False)
            nc.tensor.matmul(ps2[:], lhsT=onesrow[:], rhs=b2r[:], start=False, stop=True)
            nc.scalar.copy(s2_all[:, t * D2 : (t + 1) * D2], ps2[:])

            # split AllGather: fire quarter q as soon as its rows are written
            for qq, tq in enumerate((12, 24, 36, 48, 61, 73, 85, NTILES - 1)):
                if t == tq:
                    nc.gpsimd.collective_compute(
                        "AllGather",
                        mybir.AluOpType.bypass,
                        ins=[z2_shard[qq * QS : (qq + 1) * QS, :]],
                        outs=[z2_full[qq * N_CORES * QS : (qq + 1) * N_CORES * QS, :]],
                        replica_groups=[list(range(N_CORES))],
                    )

        # ---------- Layer 2 + readout accumulation ----------
        pro = ps_ro.tile([N_GRAPHS, D2 + 1], f32, tag="ro")
        for t in range(NTILES):
            nch_t = int(K[t])
            pagg = ps_agg.tile([P, D2], f32, tag="agg")
            for j in range(nch_t):
                ch = int(offs[t]) + j
                m = msgs.tile([P, D2], f32, tag="msg2")
                nc.gpsimd.indirect_dma_start(
                    out=m[:], out_offset=None, in_=z2_full[:, :],
                    in_offset=IndirectOffsetOnAxis(ap=gidxZ[:, ch : ch + 1], axis=0),
                )
                s = build_S(ch)
                nc.tensor.matmul(pagg[:], lhsT=s[:], rhs=m[:],
                                 start=(j == 0), stop=False)
            # add self term s2 (+b2) into the same PSUM, then ReLU from PSUM
            nc.tensor.matmul(pagg[:], lhsT=ident[:],
                             rhs=s2_all[:, t * D2 : (t + 1) * D2],
                             start=False, stop=True)
            h2e = sbuf.tile([P, D2 + 1], f32, tag="h2e")
            nc.scalar.activation(h2e[:, :D2], pagg[:], AF.Relu)
            nc.vector.memset(h2e[:, D2 : D2 + 1], 1.0)

            sg = msgs.tile([P, N_GRAPHS], f32, tag="Sg")
            nc.vector.tensor_tensor(
                out=sg[:], in0=iota64[:],
                in1=glT[:, t : t + 1].to_broadcast([P, N_GRAPHS]),
                op=mybir.AluOpType.is_equal,
            )
            nc.tensor.matmul(pro[:], lhsT=sg[:], rhs=h2e[:],
                             start=(t == 0), stop=(t == NTILES - 1))

        ro_sb = sbuf.tile([N_GRAPHS, D2 + 1], f32, tag="ro_sb")
        nc.vector.tensor_copy(ro_sb[:], pro[:])
        nc.sync.dma_start(ro_part[:], ro_sb[:])
        nc.gpsimd.collective_compute(
            "AllReduce",
            mybir.AluOpType.add,
            ins=[ro_part.opt()],
            outs=[ro_red.opt()],
            replica_groups=[list(range(N_CORES))],
        )

        # ---------- Per-graph mean + MLP (redundant on every core) ----------
        ro2 = sbuf.tile([N_GRAPHS, D2 + 1], f32, tag="ro2")
        nc.sync.dma_start(ro2[:], ro_red[:])
        cinv = sbuf.tile([N_GRAPHS, 1], f32, tag="cinv")
        nc.vector.tensor_scalar_max(cinv[:], ro2[:, D2 : D2 + 1], 1.0)
        nc.vector.reciprocal(cinv[:], cinv[:])
        hg = sbuf.tile([N_GRAPHS, D2], f32, tag="hg")
        nc.vector.tensor_scalar(
            out=hg[:], in0=ro2[:, :D2], scalar1=cinv[:, 0:1], scalar2=None,
            op0=mybir.AluOpType.mult,
        )
        ptr4 = ps_agg.tile([P, P], f32, tag="agg")
        nc.tensor.transpose(ptr4[:D2, :N_GRAPHS], hg[:], ident[:N_GRAPHS, :N_GRAPHS])
        hgT = sbuf.tile([D2, N_GRAPHS], f32, tag="hgT")
        nc.vector.tensor_copy(hgT[:], ptr4[:D2, :N_GRAPHS])

        pfc1 = ps_h.tile([N_GRAPHS, 10], f32, tag="h1T")
        nc.tensor.matmul(pfc1[:], lhsT=hgT[:], rhs=fc1w[:], start=True, stop=False)
        nc.tensor.matmul(pfc1[:], lhsT=onesrow[:, :N_GRAPHS], rhs=fc1b[:], start=False, stop=True)
        a1 = sbuf.tile([N_GRAPHS, 10], f32, tag="a1")
        nc.scalar.activation(a1[:], pfc1[:], AF.Relu)

        ptr5 = ps_agg.tile([P, P], f32, tag="agg")
        nc.tensor.transpose(ptr5[:10, :N_GRAPHS], a1[:], ident[:N_GRAPHS, :N_GRAPHS])
        a1T = sbuf.tile([10, N_GRAPHS], f32, tag="a1T")
        nc.vector.tensor_copy(a1T[:], ptr5[:10, :N_GRAPHS])

        pout = ps_z.tile([N_GRAPHS, 1], f32, tag="z2")
        nc.tensor.matmul(pout[:], lhsT=a1T[:], rhs=fc2w[:], start=True, stop=False)
        nc.tensor.matmul(pout[:], lhsT=onesrow[:, :N_GRAPHS], rhs=fc2b[:], start=False, stop=True)
        osb = sbuf.tile([N_GRAPHS, 1], f32, tag="osb")
        nc.vector.tensor_copy(osb[:], pout[:])
        nc.sync.dma_start(out_dram[:], osb[:])

    nc.compile()
    return nc


def kernel(feat, Wself1, Wneigh1, b1, Wself2, Wneigh2, b2,
           fc1_w, fc1_b, fc2_w, fc2_b, src, dst, graph_ids):
    global LAST_RESULTS
    feat = np.asarray(feat, dtype=np.float32)
    src = np.asarray(src, dtype=np.int64)
    dst = np.asarray(dst, dtype=np.int64)
    graph_ids = np.asarray(graph_ids, dtype=np.int64)

    K, offs, NCH, percore, featP = _host_prep(src, dst, graph_ids, feat)
    nc = _build_program(K, offs, NCH)

    iota128 = np.tile(np.arange(P, dtype=np.float32), (P, 1))
    iota64 = np.tile(np.arange(N_GRAPHS, dtype=np.float32), (P, 1))
    ident = np.eye(P, dtype=np.float32)
    onesrow = np.ones((1, P), dtype=np.float32)
    b1c = np.zeros((P, 1), dtype=np.float32)
    b1c[:D1, 0] = np.asarray(b1, np.float32)

    in_maps = []
    for c in range(N_CORES):
        pc = percore[c]
        in_maps.append({
            "featP": featP,
            "flocT": pc["flocT"],
            "gidxF": pc["gidxF"],
            "gidxZ": pc["gidxZ"],
            "dstl": pc["dstl"],
            "degE": pc["degE"],
            "glT": pc["glT"],
            "iota128": iota128,
            "iota64": iota64,
            "ident": ident,
            "onesrow": onesrow,
            "ws1": np.asarray(Wself1, np.float32),
            "wn1": np.asarray(Wneigh1, np.float32),
            "b1c": b1c,
            "ws2": np.asarray(Wself2, np.float32),
            "wn2": np.asarray(Wneigh2, np.float32),
            "b2r": np.asarray(b2, np.float32).reshape(1, D2),
            "fc1w": np.asarray(fc1_w, np.float32),
            "fc1b": np.asarray(fc1_b, np.float32).reshape(1, 10),
            "fc2w": np.asarray(fc2_w, np.float32),
            "fc2b": np.asarray(fc2_b, np.float32).reshape(1, 1),
        })

    res = run_bass_kernel_spmd(nc, in_maps, list(range(N_CORES)))
    LAST_RESULTS = res
    return np.asarray(res.results[0]["out"], dtype=np.float32)
